# revision 3
# baseline (speedup 1.0000x reference)
"""DETR3D-core kernel for 8 trn2 NeuronCores.

Strategy (query-sharded):
  - 1800 (batch, query) pairs are split 225-per-core (core r, batch r//4).
  - Host computes the projection geometry (reference-exact jax CPU math),
    per-sample bilinear corner indices and combined weights
    (bilinear * valid-mask / count), already folded per 2x2 patch.
  - Device: dma_gather of 2-pixel rows from channels-last camera images,
    weighted accumulation -> tgt slice, x = q_emb + tgt, PE-transpose,
    AllGather of x^T within each batch group of 4 cores, then the full
    attention + layernorm + heads for the core's 225 queries.
"""

import numpy as np

# ---- problem constants (hardcoded; must match the grader's setup_inputs) ----
B, NCAM, C, H, W = 2, 6, 256, 116, 200
Q, D, NH, HD = 900, 256, 8, 32
PIX = H * W                      # 23200 pixels per camera
QS = 225                         # queries per core
NT = 2                           # query partition-tiles per core
CNT = (128, 97)                  # valid rows per tile
QLP = 256                        # padded local queries (2 * 128)
NKT = 8
KTS = [128] * 7 + [4]            # key k-tiles covering 900
N_CORES = 8
RG = [[0, 1, 2, 3], [4, 5, 6, 7]]
SCALE = 1.0 / np.sqrt(HD)
EPS = 1e-5

_COMPILED = {}


def _build():
    import concourse.bass as bass
    import concourse.mybir as mybir
    import concourse.tile as tile
    from concourse import bacc
    from concourse.masks import make_identity

    F32 = mybir.dt.float32
    I16 = mybir.dt.int16
    AF = mybir.ActivationFunctionType
    AX = mybir.AxisListType

    nc = bacc.Bacc("TRN2", target_bir_lowering=False, debug=False,
                   num_devices=N_CORES)

    # ---- I/O ----
    imgs = [nc.dram_tensor(f"img{j}", [PIX * C], F32, kind="ExternalInput")
            for j in range(NCAM)]
    gidx = nc.dram_tensor("gidx", [NT * NCAM, 128, 16], I16,
                          kind="ExternalInput").ap()
    wts = nc.dram_tensor("wts", [NT, 128, 24], F32, kind="ExternalInput").ap()
    qsl = nc.dram_tensor("qsl", [NT, 128, D], F32, kind="ExternalInput").ap()
    wqkvT = nc.dram_tensor("wqkvT", [D, 3 * D], F32, kind="ExternalInput").ap()
    bqkv = nc.dram_tensor("bqkv", [1, 3 * D], F32, kind="ExternalInput").ap()
    wvpack = nc.dram_tensor("wvpack", [D, 264], F32, kind="ExternalInput").ap()
    bvpack = nc.dram_tensor("bvpack", [1, 264], F32, kind="ExternalInput").ap()
    wo8 = nc.dram_tensor("wo8", [32, NH, D], F32, kind="ExternalInput").ap()
    bout = nc.dram_tensor("bout", [1, D], F32, kind="ExternalInput").ap()
    wheads = nc.dram_tensor("wheads", [D, 8], F32, kind="ExternalInput").ap()
    bheads = nc.dram_tensor("bheads", [1, 8], F32, kind="ExternalInput").ap()
    normw = nc.dram_tensor("normw", [1, D], F32, kind="ExternalInput").ap()
    normb = nc.dram_tensor("normb", [1, D], F32, kind="ExternalInput").ap()
    out_cls = nc.dram_tensor("out_cls", [NT, 128, 2], F32,
                             kind="ExternalOutput").ap()
    out_box = nc.dram_tensor("out_box", [NT, 128, 6], F32,
                             kind="ExternalOutput").ap()

    with tile.TileContext(nc) as tc:
        with (
            tc.tile_pool(name="const", bufs=1) as cp,
            tc.tile_pool(name="work", bufs=2) as wp,
            tc.tile_pool(name="attn", bufs=1) as ap_,
            tc.tile_pool(name="ps_big", bufs=2, space="PSUM") as ps_big,
            tc.tile_pool(name="ps_s", bufs=4, space="PSUM") as ps_s,
            tc.tile_pool(name="ps_o", bufs=1, space="PSUM") as ps_o,
            tc.tile_pool(name="ps_t", bufs=1, space="PSUM") as ps_t,
            tc.tile_pool(name="dram", bufs=1, space="DRAM") as dp,
        ):
            # ---------------- constants ----------------
            ident = cp.tile([128, 128], F32, tag="ident")
            make_identity(nc, ident[:])
            ones = cp.tile([1, 512], F32, tag="ones")
            nc.vector.memset(ones[:], 1.0)
            ones32 = cp.tile([33, 32], F32, tag="ones32")
            nc.vector.memset(ones32[32:33, :], 1.0)
            eps_sb = cp.tile([128, 1], F32, tag="eps")
            nc.vector.memset(eps_sb[:], EPS)

            wqkvT_sb = cp.tile([128, 2, 3 * D], F32, tag="wqkvT")
            nc.sync.dma_start(wqkvT_sb[:],
                              wqkvT.rearrange("(k p) n -> p k n", p=128))
            wv_sb = cp.tile([128, 2, 264], F32, tag="wv")
            nc.sync.dma_start(wv_sb[:],
                              wvpack.rearrange("(k p) n -> p k n", p=128))
            wo8_sb = cp.tile([32, NH, D], F32, tag="wo8")
            nc.sync.dma_start(wo8_sb[:], wo8[:])
            wh_sb = cp.tile([128, 2, 8], F32, tag="wh")
            nc.sync.dma_start(wh_sb[:],
                              wheads.rearrange("(k p) n -> p k n", p=128))
            bqkv_sb = cp.tile([1, 3 * D], F32, tag="bqkv")
            nc.sync.dma_start(bqkv_sb[:], bqkv[:])
            bv_sb = cp.tile([1, 264], F32, tag="bv")
            nc.sync.dma_start(bv_sb[:], bvpack[:])
            bo_sb = cp.tile([1, D], F32, tag="bo")
            nc.sync.dma_start(bo_sb[:], bout[:])
            bh_sb = cp.tile([1, 8], F32, tag="bh")
            nc.sync.dma_start(bh_sb[:], bheads[:])
            nw_sb = cp.tile([1, D], F32, tag="nw")
            nc.sync.dma_start(nw_sb[:], normw[:])
            nb_sb = cp.tile([1, D], F32, tag="nb")
            nc.sync.dma_start(nb_sb[:], normb[:])
            nwb = cp.tile([128, D], F32, tag="nwb")
            nc.gpsimd.partition_broadcast(nwb[:], nw_sb[:], channels=128)
            nbb = cp.tile([128, D], F32, tag="nbb")
            nc.gpsimd.partition_broadcast(nbb[:], nb_sb[:], channels=128)

            gidx_sb = cp.tile([128, NT * NCAM, 16], I16, tag="gidx")
            nc.sync.dma_start(gidx_sb[:], gidx.rearrange("c p s -> p c s"))
            wts_sb = cp.tile([128, NT, 24], F32, tag="wts")
            nc.sync.dma_start(wts_sb[:], wts.rearrange("t p w -> p t w"))
            qsl_sb = cp.tile([128, NT, D], F32, tag="qsl")
            nc.sync.dma_start(qsl_sb[:], qsl.rearrange("t p d -> p t d"))

            agin_sb = ap_.tile([128, 2, QLP], F32, tag="agin")

            # ---------------- phase A: gather + weighted sum ----------------
            for t in range(NT):
                g_t = wp.tile([128, 2 * NCAM, 2 * C], F32, tag="gt")
                for j in range(NCAM):
                    nc.gpsimd.dma_gather(
                        out_ap=g_t[:, 2 * j:2 * j + 2, :],
                        in_ap=bass.AP(imgs[j], 0, [[C, PIX - 1], [1, 2 * C]]),
                        idxs_ap=gidx_sb[:, t * NCAM + j, :],
                        num_idxs=256,
                        num_idxs_reg=256,
                        elem_size=2 * C,
                        elem_step=C,
                    )
                acc = wp.tile([128, C], F32, tag="acc")
                first = True
                for j in range(NCAM):
                    for rr in range(2):
                        for cc_ in range(2):
                            k = j * 4 + rr * 2 + cc_
                            w_ap = wts_sb[:, t, k:k + 1]
                            src = g_t[:, 2 * j + rr, cc_ * C:(cc_ + 1) * C]
                            if first:
                                nc.vector.tensor_scalar_mul(acc[:], src, w_ap)
                                first = False
                            else:
                                tmp = wp.tile([128, C], F32, tag="tmp")
                                if k % 2 == 0:
                                    nc.scalar.mul(tmp[:], src, w_ap)
                                else:
                                    nc.vector.tensor_scalar_mul(tmp[:], src, w_ap)
                                nc.vector.tensor_add(acc[:], acc[:], tmp[:])
                x_sl = wp.tile([128, D], F32, tag="xsl")
                nc.vector.tensor_add(x_sl[:], acc[:], qsl_sb[:, t, :])
                for dt in range(2):
                    psT = ps_t.tile([128, 128], F32, tag="a")
                    nc.tensor.transpose(psT[:], x_sl[:, dt * 128:(dt + 1) * 128],
                                        ident[:])
                    nc.vector.tensor_copy(agin_sb[:, dt, t * 128:(t + 1) * 128],
                                          psT[:])

            # ---------------- AllGather of x^T within batch group ----------------
            agin_d = dp.tile([128, 2, QLP], F32, tag="agin_d")
            agout_d = dp.tile([4, 128, 2, QLP], F32, tag="agout_d")
            nc.gpsimd.dma_start(agin_d[:], agin_sb[:])
            nc.gpsimd.collective_compute(
                "AllGather",
                mybir.AluOpType.bypass,
                replica_groups=RG,
                ins=[agin_d.opt()],
                outs=[agout_d.opt()],
            )
            xT_sb = [ap_.tile([128, Q], F32, tag=f"xT{dt}", name=f"xT{dt}") for dt in range(2)]
            for rk in range(4):
                for dt in range(2):
                    nc.sync.dma_start(xT_sb[dt][:, rk * QS:(rk + 1) * QS],
                                      agout_d[rk, :, dt, 0:QS])

            # ---------------- K^T, Q^T, V ----------------
            KT_sb = [ap_.tile([128, Q], F32, tag=f"KT{kd}", name=f"KT{kd}") for kd in range(2)]
            QT_sb = [ap_.tile([128, QLP], F32, tag=f"QT{kd}", name=f"QT{kd}") for kd in range(2)]
            for kd in range(2):
                for (n0, nn) in ((0, 512), (512, Q - 512)):
                    ps = ps_big.tile([128, 512], F32, tag="b")
                    for ks in range(2):
                        nc.tensor.matmul(
                            ps[:, 0:nn],
                            wqkvT_sb[:, ks, D + kd * 128:D + (kd + 1) * 128],
                            xT_sb[ks][:, n0:n0 + nn],
                            start=(ks == 0), stop=False)
                    nc.tensor.matmul(
                        ps[:, 0:nn],
                        bqkv_sb[:, D + kd * 128:D + (kd + 1) * 128],
                        ones[:, 0:nn], start=False, stop=True)
                    nc.vector.tensor_copy(KT_sb[kd][:, n0:n0 + nn], ps[:, 0:nn])
                psq = ps_big.tile([128, 512], F32, tag="b")
                for ks in range(2):
                    nc.tensor.matmul(
                        psq[:, 0:QLP],
                        wqkvT_sb[:, ks, kd * 128:(kd + 1) * 128],
                        agin_sb[:, ks, :], start=(ks == 0), stop=False)
                nc.tensor.matmul(psq[:, 0:QLP],
                                 bqkv_sb[:, kd * 128:(kd + 1) * 128],
                                 ones[:, 0:QLP], start=False, stop=True)
                nc.scalar.mul(QT_sb[kd][:], psq[:, 0:QLP], SCALE)

            V_sb = ap_.tile([128, NKT, 264], F32, tag="V")
            for mt in range(NKT):
                kc = KTS[mt]
                ps = ps_big.tile([128, 512], F32, tag="b")
                for ks in range(2):
                    nc.tensor.matmul(ps[0:kc, 0:264],
                                     xT_sb[ks][:, mt * 128:mt * 128 + kc],
                                     wv_sb[:, ks, :], start=(ks == 0), stop=False)
                nc.tensor.matmul(ps[0:kc, 0:264], ones[0:1, 0:kc], bv_sb[:],
                                 start=False, stop=True)
                nc.vector.tensor_copy(V_sb[0:kc, mt, :], ps[0:kc, 0:264])

            # ---------------- attention per head ----------------
            expS = ap_.tile([128, NKT, QLP], F32, tag="expS")
            Os = [ap_.tile([33, QLP], F32, tag=f"Os{h}", name=f"Os{h}") for h in range(NH)]
            On = [ap_.tile([32, QLP], F32, tag=f"On{h}", name=f"On{h}") for h in range(NH)]
            for h in range(NH):
                kd, hh = h // 4, h % 4
                for kt in range(NKT):
                    kc = KTS[kt]
                    psS = ps_s.tile([128, QLP], F32, tag="s")
                    kw = dict(tile_position=(96, 0)) if hh == 3 else {}
                    nc.tensor.matmul(
                        psS[0:kc, :],
                        KT_sb[kd][hh * 32:(hh + 1) * 32,
                                  kt * 128:kt * 128 + kc],
                        QT_sb[kd][hh * 32:(hh + 1) * 32, :],
                        start=True, stop=True, **kw)
                    nc.scalar.activation(expS[0:kc, kt, :], psS[0:kc, :], AF.Exp)
                psO = ps_o.tile([33, QLP], F32, tag="o")
                for kt in range(NKT):
                    kc = KTS[kt]
                    nc.tensor.matmul(psO[:],
                                     V_sb[0:kc, kt, h * 33:(h + 1) * 33],
                                     expS[0:kc, kt, :],
                                     start=(kt == 0), stop=(kt == NKT - 1))
                nc.vector.tensor_copy(Os[h][:], psO[:])
                nc.vector.reciprocal(Os[h][32:33, :], Os[h][32:33, :])
                psR = ps_t.tile([128, QLP], F32, tag="a")
                nc.tensor.matmul(psR[0:32, :], ones32[32:33, :],
                                 Os[h][32:33, :], start=True, stop=True,
                                 tile_position=(32, 0))
                nc.vector.tensor_mul(On[h][:], Os[h][0:32, :], psR[0:32, :])

            # ---------------- out_proj + layernorm + heads ----------------
            for mq in range(NT):
                psF = ps_big.tile([128, 512], F32, tag="b")
                for h in range(NH):
                    nc.tensor.matmul(psF[:, 0:D],
                                     On[h][:, mq * 128:(mq + 1) * 128],
                                     wo8_sb[:, h, :],
                                     start=(h == 0), stop=False)
                nc.tensor.matmul(psF[:, 0:D], ones[0:1, 0:128], bo_sb[:],
                                 start=False, stop=True)
                mu = wp.tile([128, 1], F32, tag="mu")
                nc.vector.reduce_sum(mu[:], psF[:, 0:D], axis=AX.X)
                nc.vector.tensor_scalar_mul(mu[:], mu[:], -1.0 / D)
                xc = wp.tile([128, D], F32, tag="xc")
                nc.scalar.activation(xc[:], psF[:, 0:D], AF.Identity,
                                     bias=mu[:, 0:1], scale=1.0)
                sqd = wp.tile([128, D], F32, tag="sqd")
                vs = wp.tile([128, 1], F32, tag="vs")
                nc.scalar.activation(sqd[:], xc[:], AF.Square, accum_out=vs[:])
                sd = wp.tile([128, 1], F32, tag="sd")
                nc.scalar.activation(sd[:], vs[:], AF.Sqrt, bias=eps_sb[:, 0:1],
                                     scale=1.0 / D)
                rs = wp.tile([128, 1], F32, tag="rs")
                nc.vector.reciprocal(rs[:], sd[:])
                xn = wp.tile([128, D], F32, tag="xn")
                nc.vector.tensor_scalar_mul(xn[:], xc[:], rs[:, 0:1])
                nc.vector.tensor_mul(xn[:], xn[:], nwb[:])
                nc.vector.tensor_add(xn[:], xn[:], nbb[:])
                xnT = wp.tile([128, 2, 128], F32, tag="xnT")
                for dt in range(2):
                    psT = ps_t.tile([128, QLP], F32, tag="a")
                    nc.tensor.transpose(psT[:, 0:128],
                                        xn[:, dt * 128:(dt + 1) * 128], ident[:])
                    nc.vector.tensor_copy(xnT[:, dt, :], psT[:, 0:128])
                psH = ps_s.tile([128, QLP], F32, tag="s")
                for dt in range(2):
                    nc.tensor.matmul(psH[:, 0:8], xnT[:, dt, :],
                                     wh_sb[:, dt, :],
                                     start=(dt == 0), stop=False)
                nc.tensor.matmul(psH[:, 0:8], ones[0:1, 0:128], bh_sb[:],
                                 start=False, stop=True)
                cls_sb = wp.tile([128, 2], F32, tag="cls")
                box_sb = wp.tile([128, 6], F32, tag="box")
                nc.vector.tensor_copy(cls_sb[:], psH[:, 0:2])
                nc.scalar.activation(box_sb[:, 0:3], psH[:, 2:5], AF.Sigmoid)
                nc.vector.tensor_copy(box_sb[:, 3:6], psH[:, 5:8])
                nc.sync.dma_start(out_cls[mq], cls_sb[:])
                nc.sync.dma_start(out_box[mq], box_sb[:])

    nc.compile()
    return nc


def get_compiled():
    if "nc" not in _COMPILED:
        _COMPILED["nc"] = _build()
    return _COMPILED["nc"]


def _host_prep(image_features, lidar2img, query_embedding, ref_w, ref_b,
               in_proj_w, in_proj_b, out_proj_w, out_proj_b,
               norm_w, norm_b, cls_w, cls_b, box_w, box_b):
    """Reference-exact geometry on jax CPU + per-core input packing."""
    import jax
    import jax.numpy as jnp

    cpu = jax.devices("cpu")[0]
    with jax.default_device(cpu):
        qe = jnp.asarray(np.asarray(query_embedding, np.float32))
        q = jnp.broadcast_to(qe[None], (B, Q, D))
        ref = jax.nn.sigmoid(q @ jnp.asarray(np.asarray(ref_w, np.float32)).T
                             + jnp.asarray(np.asarray(ref_b, np.float32)))
        homo = jnp.concatenate([ref, jnp.ones_like(ref[..., :1])], axis=-1)
        p2h = jnp.einsum('bnij,bqj->bnqi',
                         jnp.asarray(np.asarray(lidar2img, np.float32)), homo)
        z = p2h[..., 2:3]
        pts2d = p2h[..., 0:2] / (jnp.abs(z) + 1e-5)
        mask = (z > 1e-5)[..., 0]
        pn = jnp.stack([pts2d[..., 0] / (W - 1) * 2 - 1,
                        pts2d[..., 1] / (H - 1) * 2 - 1], axis=-1)
        mask_t = mask.transpose(0, 2, 1)[..., None]
        pn_view = pn.reshape(B, Q, NCAM, 2)
        valid = mask_t & (jnp.max(jnp.abs(pn_view), axis=-1, keepdims=True) < 1)
        validf = valid.astype(jnp.float32)
        count = jnp.clip(jnp.sum(validf, axis=2), 1.0, None)  # [B,Q,1]
        ix = (pn[..., 0] + 1.0) * (W * 0.5) - 0.5
        iy = (pn[..., 1] + 1.0) * (H * 0.5) - 0.5
        x0 = jnp.floor(ix)
        y0 = jnp.floor(iy)
        wx1 = ix - x0
        wy1 = iy - y0
        x0i = x0.astype(jnp.int32)
        y0i = y0.astype(jnp.int32)

    x0i = np.asarray(x0i)
    y0i = np.asarray(y0i)
    wx1 = np.asarray(wx1, np.float32)
    wy1 = np.asarray(wy1, np.float32)
    vf = np.asarray(validf, np.float32)[..., 0].transpose(0, 2, 1)  # [B,N,Q]
    cnt = np.asarray(count, np.float32)[..., 0]                     # [B,Q]

    xs = np.clip(x0i, 0, W - 2)
    ys = np.clip(y0i, 0, H - 2)
    w4 = np.zeros((B, NCAM, Q, 2, 2), np.float32)
    for dy in (0, 1):
        wy = wy1 if dy else (1.0 - wy1)
        cy = y0i + dy
        for dx in (0, 1):
            wx = wx1 if dx else (1.0 - wx1)
            cx = x0i + dx
            inb = (cy >= 0) & (cy < H) & (cx >= 0) & (cx < W)
            rr = cy - ys
            cc = cx - xs
            wgt = (wy * wx).astype(np.float32)
            for r_ in (0, 1):
                for c_ in (0, 1):
                    m = inb & (rr == r_) & (cc == c_)
                    w4[:, :, :, r_, c_] += np.where(m, wgt, 0.0)
    w4 *= (vf / cnt[:, None, :])[..., None, None]
    pix = (ys[..., None] + np.arange(2)[None, None, None]) * W + xs[..., None]
    pix = pix.astype(np.int32)  # [B,N,Q,2]

    # channels-last images per batch: [N, PIX*C]
    imgs_cl = {}
    feats = np.asarray(image_features, np.float32)
    for b in range(B):
        imgs_cl[b] = np.ascontiguousarray(
            feats[b].transpose(0, 2, 3, 1)).reshape(NCAM, PIX * C)

    qe_np = np.asarray(query_embedding, np.float32)
    in_proj_w = np.asarray(in_proj_w, np.float32)
    in_proj_b = np.asarray(in_proj_b, np.float32)
    wqkvT = np.ascontiguousarray(in_proj_w.T)            # [256, 768]
    bqkv = in_proj_b[None, :].copy()
    wvpack = np.zeros((D, 264), np.float32)
    bvpack = np.zeros((1, 264), np.float32)
    for h in range(NH):
        wvpack[:, h * 33:h * 33 + 32] = wqkvT[:, 2 * D + h * 32:2 * D + (h + 1) * 32]
        bvpack[0, h * 33:h * 33 + 32] = in_proj_b[2 * D + h * 32:2 * D + (h + 1) * 32]
        bvpack[0, h * 33 + 32] = 1.0
    woT = np.ascontiguousarray(np.asarray(out_proj_w, np.float32).T)  # [256,256]
    wo8 = np.zeros((32, NH, D), np.float32)
    for h in range(NH):
        wo8[:, h, :] = woT[h * 32:(h + 1) * 32, :]
    bout = np.asarray(out_proj_b, np.float32)[None, :].copy()
    wheads = np.ascontiguousarray(
        np.concatenate([np.asarray(cls_w, np.float32),
                        np.asarray(box_w, np.float32)], axis=0).T)  # [256, 8]
    bheads = np.concatenate([np.asarray(cls_b, np.float32),
                             np.asarray(box_b, np.float32)])[None, :].copy()
    normw = np.asarray(norm_w, np.float32)[None, :].copy()
    normb = np.asarray(norm_b, np.float32)[None, :].copy()

    in_maps = []
    for r in range(N_CORES):
        b = r // 4
        qbase = (r % 4) * QS
        gidx_arr = np.zeros((NT * NCAM, 128, 16), np.int16)
        wts_arr = np.zeros((NT, 128, 24), np.float32)
        qsl_arr = np.zeros((NT, 128, D), np.float32)
        for t in range(NT):
            cnt_t = CNT[t]
            qq = qbase + t * 128 + np.arange(cnt_t)  # queries of this tile
            qsl_arr[t, :cnt_t, :] = qe_np[qq, :]
            for j in range(NCAM):
                idx_flat = np.zeros(256, np.int16)
                for r_ in (0, 1):
                    idx_flat[r_ * 128:r_ * 128 + cnt_t] = pix[b, j, qq, r_]
                wrapped = idx_flat.reshape(16, 16).T  # [i%16, i//16]
                gidx_arr[t * NCAM + j] = np.tile(wrapped, (8, 1))
                for r_ in (0, 1):
                    for c_ in (0, 1):
                        wts_arr[t, :cnt_t, j * 4 + r_ * 2 + c_] = \
                            w4[b, j, qq, r_, c_]
        m = {
            "gidx": gidx_arr, "wts": wts_arr, "qsl": qsl_arr,
            "wqkvT": wqkvT, "bqkv": bqkv, "wvpack": wvpack, "bvpack": bvpack,
            "wo8": wo8, "bout": bout, "wheads": wheads, "bheads": bheads,
            "normw": normw, "normb": normb,
        }
        for j in range(NCAM):
            m[f"img{j}"] = imgs_cl[b][j]
        in_maps.append(m)
    return in_maps


def _assemble(results):
    cls = np.zeros((B * Q, 2), np.float32)
    box = np.zeros((B * Q, 6), np.float32)
    for r in range(N_CORES):
        for t in range(NT):
            g0 = QS * r + 128 * t
            cnt_t = CNT[t]
            cls[g0:g0 + cnt_t] = results[r]["out_cls"][t, :cnt_t]
            box[g0:g0 + cnt_t] = results[r]["out_box"][t, :cnt_t]
    return cls.reshape(B, Q, 2), box.reshape(B, Q, 6)


def kernel(**inputs):
    from concourse import bass_utils
    nc = get_compiled()
    in_maps = _host_prep(**inputs)
    res = bass_utils.run_bass_kernel_spmd(nc, in_maps,
                                          core_ids=list(range(N_CORES)))
    return _assemble(res.results)
